# revision 32
# baseline (speedup 1.0000x reference)
"""Histogram-equalization kernel for Trainium2 (Bass), 8-core data parallel.

Input:  images [64, 512, 512, 3] int32 (values 0..255)
Output: [64, 512, 512, 3] uint8 (per-image per-channel equalization).

Wall-clock here is dominated by host<->device transfer and dispatch, so the
host path is organized around minimizing bytes moved and per-call overhead:

  - pixels are cast to uint8 on host (4x fewer upload bytes than int32);
  - the Bass program is compiled once and wrapped in a single cached
    jax.jit(shard_map(bass_exec)) callable (run_bass_kernel_spmd re-traces
    and re-lowers on every call, and ships 50MB of zero-filled output
    buffers per call on top of the input);
  - the batch is processed in CHUNK-image slices, dispatched
    asynchronously so host casting, uploads, device exec and downloads
    pipeline against each other.

Device side per core: 1 image per dispatch, 3 channels of 262144 px each
as [128, 2048] int16 tiles.  DVE runs in 2x mode (2 elems/lane/cycle) only
when every operand is 2-byte and packed, and tensor_reduce never does, so
all hot ops are int16 with fold-trees instead of reduces:

  Histogram: per 128-px chunk, eq16[p, b*128+f] = (x==b) against a
    materialized repeated-iota (all operands packed int16 -> 2x), then an
    in-place log2 fold over f (2x until width 2); per-channel partition
    fold via a ones-vector PE matmul into PSUM (frees ~33K DVE
    cycles/channel vs the strided one-row reduce).
  LUT derivation batched on [3, 256] fp32 tiles (exact integer math via
    round-cast + residual correction), cast to int16.
  Apply: 16x16 hi/lo nibble split, f-major one-hot slabs; the 256-term
    product tile is int16/packed (2x), folded over lo with a log2 tree;
    final hi-select products and reduce; strided uint8 write
    re-interleaves RGB.
"""

import sys

sys.path.insert(0, "/opt/trn_rl_repo")

import numpy as np

P = 128
H = W = 512
CH = 3
N_CORES = 8
CHUNK = 8  # images per dispatch (CHUNK // N_CORES per core)
F = (H * W) // P  # 2048
NPX = H * W
FH = 128  # histogram chunk width (pixels per partition per eq tile)
FA = 128  # apply chunk (prod tile [128, 16*FA*16] int16 = 64KB/part)

_cache = {}


def build(n_img, debug=False):
    from contextlib import ExitStack

    import concourse.bacc as bacc
    import concourse.mybir as mybir
    from concourse.tile import TileContext

    dt = mybir.dt
    Alu = mybir.AluOpType
    AX = mybir.AxisListType

    nch = n_img * CH
    nc = bacc.Bacc("TRN2", target_bir_lowering=False, debug=False)
    imgs = nc.dram_tensor("imgs", [n_img, H * W * CH], dt.uint8, kind="ExternalInput")
    out = nc.dram_tensor("out", [n_img, H * W * CH], dt.uint8, kind="ExternalOutput")
    dbg = None
    if debug:
        dbg = nc.dram_tensor("dbg", [nch, 256], dt.float32, kind="ExternalOutput")

    with TileContext(nc) as tc, ExitStack() as ctx:
        sb = ctx.enter_context(tc.tile_pool(name="sb", bufs=1))
        sbd = ctx.enter_context(tc.tile_pool(name="sbd", bufs=1))
        psum = ctx.enter_context(
            tc.tile_pool(name="psum", bufs=1, space="PSUM")
        )

        # constants materialized on all partitions (cm=0)
        iotaL = sb.tile([P, 16], dt.int16, tag="iotaL")
        nc.gpsimd.iota(iotaL[:], pattern=[[1, 16]], base=0, channel_multiplier=0)
        # iotaRep[p, b*FH + f] = b  (bin value repeated FH times, packed int16)
        iotaRep = sb.tile([P, 256 * FH], dt.int16, tag="iotaRep")
        nc.gpsimd.iota(
            iotaRep[:], pattern=[[1, 256], [0, FH]], base=0, channel_multiplier=0
        )
        ones128 = sb.tile([P, 1], dt.float32, tag="ones128")
        nc.vector.memset(ones128[:], 1.0)
        iotaf = sbd.tile([nch, 256], dt.float32, tag="iotaf")
        ioti = sbd.tile([nch, 256], dt.int32, tag="ioti")
        nc.gpsimd.iota(ioti[:], pattern=[[1, 256]], base=0, channel_multiplier=0)
        nc.vector.tensor_copy(iotaf[:], ioti[:])

        histos = sbd.tile([nch, 256], dt.float32, tag="histos")
        # per-channel partition folds stage on partition 0, then one DMA
        # scatters the rows to histos partitions (PE must write psum part 0)
        stageh = sb.tile([1, nch * 256], dt.float32, tag="stageh")

        # ---------- Loop 1: histograms ----------
        x16s = {}
        for img in range(n_img):
            img8 = sb.tile([P, H * W * CH // P], dt.uint8, tag=f"img8_{img}")
            nc.sync.dma_start(out=img8[:], in_=imgs[img : img + 1, :])
            for c in range(CH):
                ch = img * CH + c
                x16 = sb.tile([P, F], dt.int16, tag=f"x16_{ch}")
                x16s[ch] = x16
                nc.vector.tensor_copy(x16[:], img8[:, c::3])

                part16 = sb.tile([P, 256], dt.int16, tag="part16")
                for k in range(F // FH):
                    eq = sb.tile([P, 256 * FH], dt.int16, tag="big16")
                    # eq[p, b*FH + f] = (x16[p, k*FH + f] == b); all operands
                    # 2-byte packed -> DVE 2x mode
                    nc.vector.tensor_tensor(
                        out=eq[:],
                        in0=x16[:, k * FH : (k + 1) * FH]
                        .unsqueeze(1)
                        .to_broadcast([P, 256, FH]),
                        in1=iotaRep[:].rearrange("p (b f) -> p b f", f=FH),
                        op=Alu.is_equal,
                    )
                    # in-place fold over f: 128 -> 1 (2x until width 2)
                    eqv = eq[:].rearrange("p (b f) -> p b f", f=FH)
                    w = FH
                    while w > 1:
                        hw = w // 2
                        nc.vector.tensor_tensor(
                            out=eqv[:, :, :hw],
                            in0=eqv[:, :, :hw],
                            in1=eqv[:, :, hw:w],
                            op=Alu.add,
                        )
                        w = hw
                    cnt = eq[:, 0 :: FH]  # [P, 256] strided chunk counts
                    if k == 0:
                        nc.vector.tensor_copy(part16[:], cnt)
                    else:
                        nc.vector.tensor_tensor(
                            out=part16[:], in0=part16[:], in1=cnt, op=Alu.add
                        )
                # partition fold: ones^T @ part -> [1, 256] PSUM (PE)
                partf = sb.tile([P, 256], dt.float32, tag="partf")
                nc.vector.tensor_copy(partf[:], part16[:])
                ph = psum.tile([1, 256], dt.float32, tag=f"ph{ch % 2}")
                nc.tensor.matmul(ph[:], ones128[:], partf[:], start=True, stop=True)
                nc.vector.tensor_copy(
                    stageh[0:1, ch * 256 : (ch + 1) * 256], ph[:]
                )
        nc.sync.dma_start(out=histos[:, :], in_=stageh[:])

        # ---------- Batched LUT derivation on [nch, 256] ----------
        NC2 = nch
        ca = sbd.tile([NC2, 256], dt.float32, tag="ca")
        cb = sbd.tile([NC2, 256], dt.float32, tag="cb")
        src = histos
        for k in range(8):
            s = 1 << k
            dst = ca if (k % 2 == 0) else cb
            nc.vector.tensor_copy(dst[:, :s], src[:, :s])
            nc.vector.tensor_tensor(
                out=dst[:, s:256], in0=src[:, s:256], in1=src[:, : 256 - s],
                op=Alu.add,
            )
            src = dst
        cum = src  # cb
        t1 = ca

        nc.vector.tensor_scalar(
            out=t1[:], in0=cum[:], scalar1=float(NPX), scalar2=None, op0=Alu.is_lt
        )
        nc.vector.tensor_tensor(out=t1[:], in0=t1[:], in1=cum[:], op=Alu.mult)
        m2 = sbd.tile([NC2, 1], dt.float32, tag="m2")
        nc.vector.tensor_reduce(out=m2[:], in_=t1[:], axis=AX.X, op=Alu.max)

        stepf = sbd.tile([NC2, 1], dt.float32, tag="stepf")
        nc.vector.tensor_scalar(
            out=stepf[:], in0=m2[:], scalar1=1.0 / 255.0, scalar2=None, op0=Alu.mult
        )
        stepi = sbd.tile([NC2, 1], dt.int32, tag="stepi")
        nc.vector.tensor_copy(stepi[:], stepf[:])
        nc.vector.tensor_copy(stepf[:], stepi[:])
        se = sbd.tile([NC2, 1], dt.float32, tag="se")
        nc.vector.tensor_scalar(
            out=se[:], in0=stepf[:], scalar1=-255.0, scalar2=None, op0=Alu.mult
        )
        nc.vector.tensor_tensor(out=se[:], in0=m2[:], in1=se[:], op=Alu.add)
        scor = sbd.tile([NC2, 1], dt.float32, tag="scor")
        nc.vector.tensor_scalar(
            out=scor[:], in0=se[:], scalar1=0.0, scalar2=None, op0=Alu.is_lt
        )
        nc.vector.tensor_tensor(
            out=stepf[:], in0=stepf[:], in1=scor[:], op=Alu.subtract
        )
        nc.vector.tensor_scalar(
            out=scor[:], in0=se[:], scalar1=255.0, scalar2=None, op0=Alu.is_ge
        )
        nc.vector.tensor_tensor(out=stepf[:], in0=stepf[:], in1=scor[:], op=Alu.add)

        s_f = sbd.tile([NC2, 1], dt.float32, tag="s_f")
        nc.vector.tensor_scalar(
            out=s_f[:], in0=stepf[:], scalar1=1.0, scalar2=None, op0=Alu.max
        )
        halff = sbd.tile([NC2, 1], dt.float32, tag="halff")
        halfi = sbd.tile([NC2, 1], dt.int32, tag="halfi")
        nc.vector.tensor_scalar(
            out=halff[:], in0=s_f[:], scalar1=0.5, scalar2=-0.25,
            op0=Alu.mult, op1=Alu.add,
        )
        nc.vector.tensor_copy(halfi[:], halff[:])
        nc.vector.tensor_copy(halff[:], halfi[:])

        r0 = sbd.tile([NC2, 1], dt.float32, tag="r0")
        nc.vector.reciprocal(r0[:], s_f[:])
        tn = sbd.tile([NC2, 1], dt.float32, tag="tn")
        nc.vector.tensor_tensor(out=tn[:], in0=s_f[:], in1=r0[:], op=Alu.mult)
        nc.vector.tensor_scalar(
            out=tn[:], in0=tn[:], scalar1=-1.0, scalar2=2.0, op0=Alu.mult, op1=Alu.add
        )
        r1 = sbd.tile([NC2, 1], dt.float32, tag="r1")
        nc.vector.tensor_tensor(out=r1[:], in0=r0[:], in1=tn[:], op=Alu.mult)

        csp = sbd.tile([NC2, 256], dt.float32, tag="csp")
        nc.vector.memset(csp[:, :1], 0.0)
        nc.vector.tensor_copy(csp[:, 1:256], cum[:, :255])

        num = sbd.tile([NC2, 256], dt.float32, tag="num")
        nc.vector.tensor_scalar(
            out=num[:], in0=csp[:], scalar1=halff[:, :1], scalar2=r1[:, :1],
            op0=Alu.add, op1=Alu.mult,
        )
        q0i = sbd.tile([NC2, 256], dt.int32, tag="q0i")
        nc.vector.tensor_copy(q0i[:], num[:])
        q0 = sbd.tile([NC2, 256], dt.float32, tag="q0")
        nc.vector.tensor_copy(q0[:], q0i[:])

        e = sbd.tile([NC2, 256], dt.float32, tag="e")
        nc.vector.tensor_scalar(
            out=e[:], in0=q0[:], scalar1=s_f[:, :1], scalar2=None, op0=Alu.mult
        )
        nc.vector.tensor_tensor(out=e[:], in0=csp[:], in1=e[:], op=Alu.subtract)
        nc.vector.tensor_scalar(
            out=e[:], in0=e[:], scalar1=halff[:, :1], scalar2=None, op0=Alu.add
        )
        corr = sbd.tile([NC2, 256], dt.float32, tag="corr")
        nc.vector.tensor_scalar(
            out=corr[:], in0=e[:], scalar1=s_f[:, :1], scalar2=None, op0=Alu.is_ge
        )
        nc.vector.tensor_tensor(out=q0[:], in0=q0[:], in1=corr[:], op=Alu.add)
        nc.vector.tensor_scalar(
            out=corr[:], in0=e[:], scalar1=0.0, scalar2=None, op0=Alu.is_lt
        )
        nc.vector.tensor_tensor(out=q0[:], in0=q0[:], in1=corr[:], op=Alu.subtract)
        nc.vector.tensor_scalar(
            out=q0[:], in0=q0[:], scalar1=0.0, scalar2=255.0, op0=Alu.max, op1=Alu.min
        )

        m0 = sbd.tile([NC2, 1], dt.float32, tag="m0")
        nc.vector.tensor_scalar(
            out=m0[:], in0=stepf[:], scalar1=0.0, scalar2=None, op0=Alu.is_equal
        )
        lut = sbd.tile([NC2, 256], dt.float32, tag="lut")
        nc.vector.tensor_tensor(out=lut[:], in0=iotaf[:], in1=q0[:], op=Alu.subtract)
        nc.vector.tensor_scalar(
            out=lut[:], in0=lut[:], scalar1=m0[:, :1], scalar2=None, op0=Alu.mult
        )
        nc.vector.tensor_tensor(out=lut[:], in0=lut[:], in1=q0[:], op=Alu.add)
        lutb = sbd.tile([NC2, 256], dt.int16, tag="lutb")
        nc.vector.tensor_copy(lutb[:], lut[:])
        if debug:
            nc.sync.dma_start(out=dbg[:, :], in_=lut[:])

        # ---------- Loop 2: apply ----------
        for img in range(n_img):
            org = sb.tile([P, CH * F], dt.uint8, tag="org")
            for c in range(CH):
                ch = img * CH + c
                x16 = x16s[ch]
                lo16 = sb.tile([P, F], dt.int16, tag="lo16")
                hi16 = sb.tile([P, F], dt.int16, tag="hi16")
                nc.vector.tensor_scalar(
                    out=lo16[:], in0=x16[:], scalar1=15, scalar2=None,
                    op0=Alu.bitwise_and,
                )
                nc.vector.tensor_scalar(
                    out=hi16[:], in0=x16[:], scalar1=4, scalar2=None,
                    op0=Alu.logical_shift_right,
                )
                # replicate this channel's lut row to all partitions (int16)
                T16 = sb.tile([P, 256], dt.int16, tag="T16")
                nc.sync.dma_start(
                    out=T16[:],
                    in_=lutb[ch : ch + 1, :].unsqueeze(1).to_broadcast([1, P, 256]),
                )
                outb = sb.tile([P, F], dt.uint8, tag="outb")
                for k in range(F // FA):
                    sl = slice(k * FA, (k + 1) * FA)
                    # f-major slabs: slab[p, f*16 + l] = (nib[p, f] == l)
                    slabL = sb.tile([P, FA * 16], dt.int16, tag="slabL")
                    nc.vector.tensor_tensor(
                        out=slabL[:],
                        in0=lo16[:, sl].unsqueeze(2).to_broadcast([P, FA, 16]),
                        in1=iotaL[:].unsqueeze(1).to_broadcast([P, FA, 16]),
                        op=Alu.is_equal,
                    )
                    slabH = sb.tile([P, FA * 16], dt.int16, tag="slabH")
                    nc.vector.tensor_tensor(
                        out=slabH[:],
                        in0=hi16[:, sl].unsqueeze(2).to_broadcast([P, FA, 16]),
                        in1=iotaL[:].unsqueeze(1).to_broadcast([P, FA, 16]),
                        op=Alu.is_equal,
                    )
                    # prod[p, (h, f, l)] = slabL[p, (f, l)] * T16[p, (h, l)]
                    # (all operands int16 with packed l -> 2x mode)
                    prod = sb.tile([P, 16 * FA * 16], dt.int16, tag="big16")
                    nc.vector.tensor_tensor(
                        out=prod[:],
                        in0=slabL[:]
                        .rearrange("p (f l) -> p f l", l=16)
                        .unsqueeze(1)
                        .to_broadcast([P, 16, FA, 16]),
                        in1=T16[:]
                        .rearrange("p (h l) -> p h l", l=16)
                        .unsqueeze(2)
                        .to_broadcast([P, 16, FA, 16]),
                        op=Alu.mult,
                    )
                    # W[p, (h, f)] = sum_l prod: in-place fold over l
                    pv = prod[:].rearrange("p (hf l) -> p hf l", l=16)
                    w = 16
                    while w > 2:
                        hw = w // 2
                        nc.vector.tensor_tensor(
                            out=pv[:, :, :hw],
                            in0=pv[:, :, :hw],
                            in1=pv[:, :, hw:w],
                            op=Alu.add,
                        )
                        w = hw
                    Wc = sb.tile([P, 16 * FA], dt.int16, tag="Wc")
                    nc.vector.tensor_tensor(
                        out=Wc[:], in0=prod[:, 0::16], in1=prod[:, 1::16], op=Alu.add
                    )
                    # prod2[p, (f, h)] = slabH[p, (f, h)] * W[p, (h, f)]-viewed
                    prod2 = sb.tile([P, FA * 16], dt.int16, tag="prod2")
                    nc.vector.tensor_tensor(
                        out=prod2[:],
                        in0=slabH[:].rearrange("p (f h) -> p f h", h=16),
                        in1=Wc[:].rearrange("p (h f) -> p f h", h=16),
                        op=Alu.mult,
                    )
                    with nc.allow_low_precision(
                        reason="one nonzero int16 term per sum; result <= 255"
                    ):
                        nc.vector.tensor_reduce(
                            out=outb[:, sl],
                            in_=prod2[:].rearrange("p (f h) -> p f h", h=16),
                            axis=AX.X,
                            op=Alu.add,
                        )
                # interleave into RGB layout (strided uint8 write)
                nc.vector.tensor_copy(org[:, c :: CH], outb[:])
            nc.sync.dma_start(out=out[img : img + 1, :], in_=org[:])

    nc.compile()
    return nc


def _get_fn():
    """Build the Bass program once and wrap it in a cached
    jax.jit(shard_map(bass_exec)) callable (the same lowering path
    run_bass_kernel_spmd takes under axon, minus its per-call re-trace,
    re-lower and zero-output upload)."""
    if "fn" in _cache:
        return _cache["fn"]

    import jax
    from jax.experimental.shard_map import shard_map
    from jax.sharding import Mesh, NamedSharding, PartitionSpec
    from concourse.bass2jax import (
        _bass_exec_p,
        install_neuronx_cc_hook,
        partition_id_tensor,
    )

    install_neuronx_cc_hook()

    n_img = CHUNK // N_CORES
    nc = build(n_img)
    out_avals = (jax.core.ShapedArray((n_img, H * W * CH), np.uint8),)

    def _body(imgs):
        outs = _bass_exec_p.bind(
            imgs,
            partition_id_tensor(),
            out_avals=out_avals,
            in_names=("imgs", "partition_id"),
            out_names=("out",),
            lowering_input_output_aliases=(),
            sim_require_finite=True,
            sim_require_nnan=True,
            nc=nc,
        )
        return outs[0]

    devices = jax.devices()[:N_CORES]
    mesh = Mesh(np.asarray(devices), ("core",))
    sharding = NamedSharding(mesh, PartitionSpec("core"))

    def _make_jit():
        return jax.jit(
            shard_map(
                _body,
                mesh=mesh,
                in_specs=(PartitionSpec("core"),),
                out_specs=PartitionSpec("core"),
                check_rep=False,
            )
        )

    # AOT-compile on the C++ fast-dispatch path (no effect tokens); fall
    # back to plain jit if the fast path is unavailable in this jax version
    try:
        from concourse.bass2jax import fast_dispatch_compile

        x_spec = jax.ShapeDtypeStruct(
            (CHUNK, H * W * CH), np.uint8, sharding=sharding
        )
        sharded = fast_dispatch_compile(lambda: _make_jit().lower(x_spec).compile())
    except Exception:
        sharded = _make_jit()

    _cache["fn"] = (sharded, sharding)
    return _cache["fn"]


def kernel(images: np.ndarray) -> np.ndarray:
    import jax

    fn, sharding = _get_fn()
    images = np.asarray(images)
    B = images.shape[0]

    futs = []
    for s in range(0, B, CHUNK):
        u8 = images[s : s + CHUNK].astype(np.uint8).reshape(CHUNK, -1)
        d = jax.device_put(u8, sharding)
        r = fn(d)
        try:
            r.copy_to_host_async()
        except Exception:
            pass
        futs.append(r)

    out = np.empty((B, H * W * CH), np.uint8)
    for i, r in enumerate(futs):
        out[i * CHUNK : (i + 1) * CHUNK] = np.asarray(r)
    return out.reshape(B, H, W, CH)
